# revision 2
# baseline (speedup 1.0000x reference)
"""Bahdanau attention TRN2 kernel, v2.

Reference computation (B=64, S=1024, H=1024, fp32 inputs):
    dh_proj = decoder_hidden @ W1.T                  # [B, H]
    enc_proj = encoder_outputs @ W2.T                # [B, S, H]
    energy = tanh(dh_proj[:, None, :] + enc_proj)    # [B, S, H]
    scores = energy @ v ; attn = softmax(scores)     # [B, S]
    context = attn @ encoder_outputs                 # [B, H]

Sharding: batch across 8 cores (8 batches/core), weights replicated.

v2 design vs v1:
  - enc arrives from the host BOTH natural (bf16, for context) and
    transposed (for the projection matmul) — no PE transposes, no DVE
    PSUM->SBUF copies.
  - mode "fp8c3": the projection matmul runs in fp8e4 DoubleRow perf mode
    (2 k-blocks per instruction, 0.5 c/row) using an error-compensated
    3-chain scheme:  enc@W2 ~= eh@wh + el@whd4 + eh@wl  with
      eh = q8(enc*4), el = q8((enc*4 - eh)*4),
      wh = q8(W2*16), wl = q8(W2*16 - wh), whd4 = q8(wh/4),
    every product at scale 64, folded back via the tanh input scale 1/64.
    The x16 scale on W2 keeps its fp8 mantissas in the normal range
    (38% of raw W2 is subnormal in e4m3).  Simulated end-to-end error:
    attn 2.3e-3 / ctx 2.7e-3 (gate is 2e-2).
  - mode "bf16": plain bf16 projection matmul (1 c/row), error ~3e-3.
  - energy is written bf16; scores = v . energy on PE in bf16; context
    from natural bf16 enc with bf16 attnT; softmax skips max-subtraction
    (scores are O(5), exp in fp32 is safe).
"""

import numpy as np
import ml_dtypes

import concourse.tile as tile
from concourse import bacc, mybir
from concourse.bass_utils import run_bass_kernel_spmd

F32 = mybir.dt.float32
F8 = mybir.dt.float8e4
BF16 = mybir.dt.bfloat16
AF = mybir.ActivationFunctionType
DR = mybir.MatmulPerfMode.DoubleRow
DRSW = mybir.MatmulPerfMode.DoubleRowSwInterleave

P = 128
N_CORES = 8
MODE = "v3"


def build_nc(b_c=8, s=1024, h=1024, iters=1, mode=None, tune=None):
    mode = mode or MODE
    assert h == 1024 and s % 512 == 0
    HB = h // P
    OB = h // P
    n_sup = s // 512
    n_chunk = s // P

    nc = bacc.Bacc("TRN2", target_bir_lowering=False, debug=False,
                   num_devices=N_CORES)

    ncols = b_c * n_sup * HB * 512
    W = HB * h
    F16 = mybir.dt.float16
    if mode in ("fp8c3", "fp8sw"):
        ehs = nc.dram_tensor("ehs", [P, ncols], F8, kind="ExternalInput").ap()
        els = nc.dram_tensor("els", [P, ncols], F8, kind="ExternalInput").ap()
        whs = nc.dram_tensor("whs", [P, W], F8, kind="ExternalInput").ap()
        wls = nc.dram_tensor("wls", [P, W], F8, kind="ExternalInput").ap()
        wds = nc.dram_tensor("wds", [P, W], F8, kind="ExternalInput").ap()
    else:
        ets = nc.dram_tensor("ets", [P, ncols], BF16, kind="ExternalInput").ap()
        wbs = nc.dram_tensor("wbs", [P, W], BF16, kind="ExternalInput").ap()
    ndt = F16 if mode == "v3" else BF16
    encn = nc.dram_tensor("encn", [b_c * s, h], ndt, kind="ExternalInput").ap()
    w1t = nc.dram_tensor("w1t", [h, h], F32, kind="ExternalInput").ap()
    dht = nc.dram_tensor("dht", [h, b_c], F32, kind="ExternalInput").ap()
    vdt = F32 if mode == "v3" else BF16
    vt = nc.dram_tensor("vt", [P, HB], vdt, kind="ExternalInput").ap()
    ctx_out = nc.dram_tensor("ctx", [b_c, h], F32, kind="ExternalOutput").ap()
    attn_out = nc.dram_tensor("attn", [b_c, s], F32, kind="ExternalOutput").ap()

    tn = {"encn": n_chunk + 6, "eh": 2, "en": 8, "pj": 2, "sc": 1, "cx": 1,
          "rows": 2}
    if mode in ("bf16", "v3"):
        tn.update({"eh": 3})
    tn.update(tune or {})

    with tile.TileContext(nc) as tc:
        from contextlib import ExitStack
        with ExitStack() as st:
            const_pool = st.enter_context(tc.tile_pool(name="const", bufs=1))
            one_t = const_pool.tile([1, 1], F32)
            nc.gpsimd.memset(one_t, 1.0)
            vt_sb = const_pool.tile([P, HB], vdt)
            nc.gpsimd.dma_start(vt_sb, vt)
            if mode == "v3":
                ones_f32 = const_pool.tile([P, 1], F32)
                nc.gpsimd.memset(ones_f32, 1.0)
                ones_f16 = const_pool.tile([P, 1], F16)
                nc.vector.tensor_copy(ones_f16, ones_f32)

            # ---- weights (resident)
            if mode in ("fp8c3", "fp8sw"):
                pass
            if mode in ("fp8c3", "fp8sw"):
                whs_t = const_pool.tile([P, HB, h], F8)
                nc.sync.dma_start(whs_t, whs.rearrange("p (k m) -> p k m", k=HB))
                wds_t = const_pool.tile([P, HB, h], F8)
                nc.sync.dma_start(wds_t, wds.rearrange("p (k m) -> p k m", k=HB))
                wls_t = const_pool.tile([P, HB, h], F8)
                nc.sync.dma_start(wls_t, wls.rearrange("p (k m) -> p k m", k=HB))
            else:
                wbs_t = const_pool.tile([P, HB, h], BF16)
                nc.sync.dma_start(wbs_t, wbs.rearrange("p (k m) -> p k m", k=HB))

            # ---- dh_projT[o][:, b] = (W1 @ dh_b)[o-block]  (fp32, once)
            dhp_pool = st.enter_context(tc.tile_pool(name="dhproj", bufs=OB))
            dh_projT = []
            hh = h // 2
            with tc.tile_pool(name="ph0", bufs=2 * HB + HB) as ph0, \
                 tc.tile_pool(name="ph0ps", bufs=2, space="PSUM") as ph0ps:
                dht_sb = []
                for k in range(HB):
                    t = ph0.tile([P, b_c], F32, tag="dh", bufs=HB)
                    nc.gpsimd.dma_start(t, dht[k * P:(k + 1) * P, :])
                    dht_sb.append(t)
                w1t_half = [[], []]
                for half in range(2):
                    for k in range(HB):
                        t = ph0.tile([P, hh], F32, tag="w1")
                        nc.sync.dma_start(t, w1t[k * P:(k + 1) * P,
                                                 half * hh:(half + 1) * hh])
                        w1t_half[half].append(t)
                for o in range(OB):
                    ps = ph0ps.tile([P, b_c], F32, tag="dhps")
                    for k in range(HB):
                        nc.tensor.matmul(
                            ps,
                            lhsT=w1t_half[o // 4][k][:, (o % 4) * P:(o % 4 + 1) * P],
                            rhs=dht_sb[k],
                            start=(k == 0), stop=(k == HB - 1))
                    t = dhp_pool.tile([P, b_c], F32, tag="dhp")
                    nc.vector.tensor_copy(t, ps)
                    dh_projT.append(t)

            # ---- pools for the steady-state loop
            encn_pool = st.enter_context(tc.tile_pool(name="encn", bufs=tn["encn"]))
            eh_pool = st.enter_context(tc.tile_pool(name="eh", bufs=tn["eh"]))
            el_pool = st.enter_context(tc.tile_pool(name="el", bufs=tn["eh"]))
            en_pool = st.enter_context(tc.tile_pool(name="energy", bufs=tn["en"]))
            row_pool = st.enter_context(tc.tile_pool(name="rows", bufs=tn["rows"]))
            sm_pool = st.enter_context(tc.tile_pool(name="small", bufs=8))
            pj_ps = st.enter_context(tc.tile_pool(name="pj_ps", bufs=tn["pj"], space="PSUM"))
            sc_ps = st.enter_context(tc.tile_pool(name="sc_ps", bufs=tn["sc"], space="PSUM"))
            cx_ps = st.enter_context(tc.tile_pool(name="cx_ps", bufs=tn["cx"], space="PSUM"))

            pending_flush = [None]
            for b in [bb for _ in range(iters) for bb in range(b_c)]:
                enc_tiles = []
                exp_row = row_pool.tile([1, s], F32, tag="exp")
                zpart = sm_pool.tile([1, n_sup], F32, tag="z")
                if mode in ("fp8c3", "fp8sw"):
                    pm = DR if mode == "fp8c3" else DRSW
                    boff = b * HB * s
                    eh_t = eh_pool.tile([P, HB, s], F8, tag="eht")
                    nc.sync.dma_start(
                        eh_t, ehs[:, boff:boff + HB * s].rearrange(
                            "p (k c) -> p k c", k=HB))
                    el_t = el_pool.tile([P, HB, s], F8, tag="elt")
                    nc.sync.dma_start(
                        el_t, els[:, boff:boff + HB * s].rearrange(
                            "p (k c) -> p k c", k=HB))
                    for j in range(n_chunk):
                        r0 = b * s + j * P
                        t = encn_pool.tile([P, h], BF16, tag="encn")
                        nc.gpsimd.dma_start(t, encn[r0:r0 + P, :])
                        enc_tiles.append(t)

                    chains = (whs_t, eh_t), (wds_t, el_t), (wls_t, eh_t)
                    for sup in range(n_sup):
                        if pending_flush[0] is not None:
                            pending_flush[0]()
                            pending_flush[0] = None

                        scp = sc_ps.tile([1, 512], F32, tag=f"scp{sup}",
                                         name=f"scp{sup}")
                        ens = []

                        def emit_score(o, scp=scp, ens=ens):
                            nc.tensor.matmul(scp, lhsT=vt_sb[:, o:o + 1],
                                             rhs=ens[o],
                                             start=(o == 0), stop=(o == OB - 1))

                        for o in range(OB):
                            pj = pj_ps.tile([P, 512], F32, tag="pj")
                            for ci, (W_, E_) in enumerate(chains):
                                for j in range(HB // 2):
                                    nc.tensor.matmul(
                                        pj,
                                        lhsT=W_[:, 2 * j:2 * j + 2,
                                                o * P:(o + 1) * P],
                                        rhs=E_[:, 2 * j:2 * j + 2,
                                               sup * 512:(sup + 1) * 512],
                                        start=(ci == 0 and j == 0),
                                        stop=(ci == 2 and j == HB // 2 - 1),
                                        perf_mode=pm)
                            en = en_pool.tile([P, 512], BF16, tag="en")
                            nc.scalar.activation(en, pj, AF.Tanh,
                                                 bias=dh_projT[o][:, b:b + 1],
                                                 scale=1.0 / 64)
                            ens.append(en)
                            if o >= 1:
                                emit_score(o - 1)

                        def flush(emit_score=emit_score, sup=sup, scp=scp,
                                  exp_row=exp_row, zpart=zpart):
                            emit_score(OB - 1)
                            nc.scalar.activation(
                                exp_row[:, sup * 512:(sup + 1) * 512],
                                scp, AF.Exp, accum_out=zpart[:, sup:sup + 1])

                        pending_flush[0] = flush
                else:
                  for sup in range(n_sup):
                    off = ((b * n_sup + sup) * HB) * 512
                    et_t = eh_pool.tile([P, HB, 512], BF16, tag="ett")
                    nc.sync.dma_start(
                        et_t, ets[:, off:off + HB * 512].rearrange(
                            "p (k c) -> p k c", k=HB))
                    for j in range(4):
                        r0 = (b * s + sup * 512 + j * P)
                        t = encn_pool.tile([P, h], ndt, tag="encn")
                        nc.gpsimd.dma_start(t, encn[r0:r0 + P, :])
                        enc_tiles.append(t)

                    # flush previous chunk's deferred score+exp tail
                    if pending_flush[0] is not None:
                        pending_flush[0]()
                        pending_flush[0] = None

                    scp = sc_ps.tile([1, 512], F32, tag="scp")
                    ens = []
                    accs = []

                    def emit_score(o, scp=scp, ens=ens):
                        nc.tensor.matmul(scp, lhsT=vt_sb[:, o:o + 1],
                                         rhs=ens[o],
                                         start=(o == 0), stop=(o == OB - 1))

                    for o in range(OB):
                        pj = pj_ps.tile([P, 512], F32, tag="pj")
                        for k in range(HB):
                            nc.tensor.matmul(
                                pj,
                                lhsT=wbs_t[:, k, o * P:(o + 1) * P],
                                rhs=et_t[:, k, :],
                                start=(k == 0), stop=(k == HB - 1))
                        en = en_pool.tile([P, 512],
                                          F16 if mode == "v3" else BF16,
                                          tag="en")
                        nc.scalar.activation(en, pj, AF.Tanh,
                                             bias=dh_projT[o][:, b:b + 1],
                                             scale=1.0)
                        ens.append(en)
                        if mode == "v3":
                            # v-weighted o-accumulation on DVE (fp16)
                            acc = en_pool.tile([P, 512], F16,
                                               tag=f"sacc{o % 2}",
                                               name=f"sacc{o % 2}", bufs=2)
                            if o == 0:
                                nc.vector.tensor_scalar_mul(
                                    acc, en, vt_sb[:, 0:1])
                            else:
                                nc.vector.scalar_tensor_tensor(
                                    acc, en, vt_sb[:, o:o + 1], accs[-1],
                                    op0=mybir.AluOpType.mult,
                                    op1=mybir.AluOpType.add)
                            accs.append(acc)
                        elif o >= 1:
                            emit_score(o - 1)

                    if mode == "v3":
                        def flush(acc=accs[-1], sup=sup, scp=scp,
                                  exp_row=exp_row, zpart=zpart):
                            nc.tensor.matmul(scp, lhsT=ones_f16, rhs=acc,
                                             start=True, stop=True)
                            nc.scalar.activation(
                                exp_row[:, sup * 512:(sup + 1) * 512],
                                scp, AF.Exp, accum_out=zpart[:, sup:sup + 1])
                    else:
                        def flush(emit_score=emit_score, sup=sup, scp=scp,
                                  exp_row=exp_row, zpart=zpart):
                            emit_score(OB - 1)
                            nc.scalar.activation(
                                exp_row[:, sup * 512:(sup + 1) * 512],
                                scp, AF.Exp, accum_out=zpart[:, sup:sup + 1])

                    pending_flush[0] = flush

                # ---- batch epilogue: softmax normalize + context
                if pending_flush[0] is not None:
                    pending_flush[0]()
                    pending_flush[0] = None
                zsum = sm_pool.tile([1, 1], F32, tag="zs")
                nc.vector.tensor_reduce(zsum, zpart, axis=mybir.AxisListType.X,
                                        op=mybir.AluOpType.add)
                invz = sm_pool.tile([1, 1], F32, tag="iz")
                nc.vector.reciprocal(invz, zsum)
                attn_row = row_pool.tile([1, s], F32, tag="attn")
                nc.vector.tensor_scalar_mul(attn_row, exp_row, invz)
                nc.sync.dma_start(attn_out[b:b + 1, :], attn_row)

                # attnT chunks [s=128, 1] via K=1 matmuls (exp, unnormalized)
                atp = sc_ps.tile([P, n_chunk], F32, tag="scp0", name="atp")
                for cc in range(n_chunk):
                    nc.tensor.matmul(atp[:, cc:cc + 1],
                                     lhsT=exp_row[:, cc * P:(cc + 1) * P],
                                     rhs=one_t,
                                     start=(cc == 0), stop=(cc == n_chunk - 1))
                attnT = sm_pool.tile([P, n_chunk],
                                     F32 if mode == "v3" else BF16, tag="at")
                nc.vector.tensor_copy(attnT, atp)

                cxp = cx_ps.tile([1, h], F32, tag="cxp")
                if mode == "v3":
                    # attn-weighted s-accumulation on DVE (fp16), then a
                    # single ones-matmul per 512-half for the partition sum
                    caccs = []
                    for cc in range(n_chunk):
                        cacc = en_pool.tile([P, h], F16, tag=f"cacc{cc % 2}",
                                            name=f"cacc{cc % 2}", bufs=2)
                        if cc == 0:
                            nc.vector.tensor_scalar_mul(
                                cacc, enc_tiles[0], attnT[:, 0:1])
                        else:
                            nc.vector.scalar_tensor_tensor(
                                cacc, enc_tiles[cc], attnT[:, cc:cc + 1],
                                caccs[-1],
                                op0=mybir.AluOpType.mult,
                                op1=mybir.AluOpType.add)
                        caccs.append(cacc)
                    for hf in range(2):
                        nc.tensor.matmul(
                            cxp[:, hf * 512:(hf + 1) * 512],
                            lhsT=ones_f16,
                            rhs=caccs[-1][:, hf * 512:(hf + 1) * 512],
                            start=True, stop=True)
                else:
                    for cc in range(n_chunk):
                        for hf in range(2):
                            nc.tensor.matmul(
                                cxp[:, hf * 512:(hf + 1) * 512],
                                lhsT=attnT[:, cc:cc + 1],
                                rhs=enc_tiles[cc][:, hf * 512:(hf + 1) * 512],
                                start=(cc == 0), stop=(cc == n_chunk - 1))
                ctx_row = row_pool.tile([1, h], F32, tag="ctx")
                nc.vector.tensor_scalar_mul(ctx_row, cxp, invz)
                nc.sync.dma_start(ctx_out[b:b + 1, :], ctx_row)

    nc.compile()
    return nc


_NC_CACHE = {}


def _get_nc(b_c=8, s=1024, h=1024, mode=None):
    key = (b_c, s, h, mode or MODE)
    if key not in _NC_CACHE:
        _NC_CACHE[key] = build_nc(b_c, s, h, mode=mode)
    return _NC_CACHE[key]


def _q8(x):
    return x.astype(ml_dtypes.float8_e4m3)


def make_in_maps(decoder_hidden, encoder_outputs, W1, W2, v, n_cores=N_CORES,
                 mode=None):
    mode = mode or MODE
    B, S, H = encoder_outputs.shape
    b_c = B // n_cores
    HB = H // P
    n_sup = S // 512
    enc = np.asarray(encoder_outputs, np.float32)
    W1 = np.asarray(W1, np.float32)
    W2 = np.asarray(W2, np.float32)
    v = np.asarray(v, np.float32)

    w1t = np.ascontiguousarray(W1.T)
    vdt = np.float32 if mode == "v3" else ml_dtypes.bfloat16
    vt = np.ascontiguousarray(v.reshape(HB, P).T.astype(vdt))

    def wlayout(wmat):
        # [o*128+m, k*128+p] -> [p, k*1024 + o*128 + m]
        a = np.asarray(wmat).reshape(OB_, P, HB, P)  # [o, m, k, p]
        out = np.ascontiguousarray(a.transpose(3, 2, 0, 1).reshape(P, HB * H))
        if mode != "fp8sw":
            return out
        # DoubleRowSwInterleave: per (k-pair, o-block) the 2x128 stationary
        # span holds [A127, B127, A126, B126, ...] (A = even k, B = odd k,
        # columns reversed).
        b4 = out.reshape(P, HB // 2, 2, OB_, P)  # [p, kp, t(k-in-pair), o, m]
        flat = np.empty((P, HB // 2, OB_, 2 * P), dtype=out.dtype)
        flat[:, :, :, 0::2] = b4[:, :, 0, :, ::-1]
        flat[:, :, :, 1::2] = b4[:, :, 1, :, ::-1]
        # back to the tile's [k, m-cols] addressing: [p, kp, t, o, m]
        fl2 = flat.reshape(P, HB // 2, OB_, 2, P).transpose(0, 1, 3, 2, 4)
        return np.ascontiguousarray(fl2.reshape(P, HB * H))

    OB_ = H // P
    if mode in ("fp8c3", "fp8sw"):
        wh8 = _q8(W2 * 16)
        wh = wh8.astype(np.float32)
        whs = wlayout(wh8)
        wls = wlayout(_q8(W2 * 16 - wh))
        wds = wlayout(_q8(wh / 4))
    else:
        wbs = wlayout(W2.astype(ml_dtypes.bfloat16))

    in_maps = []
    for i in range(n_cores):
        sl = slice(i * b_c, (i + 1) * b_c)
        encc = enc[sl]
        ndt = np.float16 if mode == "v3" else ml_dtypes.bfloat16
        m = {
            "encn": np.ascontiguousarray(
                encc.reshape(b_c * S, H).astype(ndt)),
            "w1t": w1t,
            "dht": np.ascontiguousarray(
                np.asarray(decoder_hidden[sl], np.float32).T),
            "vt": vt,
        }

        def elayout(x8):
            # fp8: [b, sup*512+c, k*128+p] -> [p, ((b*HB+k)*nsup+sup)*512+c]
            # (per-batch [k, s] tiles, k-major)
            a = x8.reshape(b_c, n_sup, 512, HB, P)  # [b, sup, c, k, p]
            return np.ascontiguousarray(
                a.transpose(4, 0, 3, 1, 2).reshape(P, b_c * n_sup * HB * 512))

        def elayout_sup(x8):
            # bf16: [b, sup*512+c, k*128+p] -> [p, ((b*nsup+sup)*HB+k)*512+c]
            a = x8.reshape(b_c, n_sup, 512, HB, P)  # [b, sup, c, k, p]
            return np.ascontiguousarray(
                a.transpose(4, 0, 1, 3, 2).reshape(P, b_c * n_sup * HB * 512))

        if mode in ("fp8c3", "fp8sw"):
            eh8 = _q8(encc * 4)
            m["ehs"] = elayout(eh8)
            m["els"] = elayout(_q8((encc * 4 - eh8.astype(np.float32)) * 4))
            m["whs"], m["wls"], m["wds"] = whs, wls, wds
        else:
            m["ets"] = elayout_sup(encc.astype(ml_dtypes.bfloat16))
            m["wbs"] = wbs
        in_maps.append(m)
    return in_maps


def kernel(decoder_hidden, encoder_outputs, W1, W2, v):
    decoder_hidden = np.asarray(decoder_hidden)
    encoder_outputs = np.asarray(encoder_outputs)
    B, S, H = encoder_outputs.shape
    b_c = B // N_CORES
    nc = _get_nc(b_c, S, H)
    in_maps = make_in_maps(decoder_hidden, encoder_outputs, W1, W2, v)
    res = run_bass_kernel_spmd(nc, in_maps, list(range(N_CORES)))
    context = np.concatenate([res.results[i]["ctx"] for i in range(N_CORES)], axis=0)
    attn = np.concatenate([res.results[i]["attn"] for i in range(N_CORES)], axis=0)
    return (context.astype(np.float32), attn.astype(np.float32))
